# revision 6
# baseline (speedup 1.0000x reference)
"""Trainium2 Bass kernel: caching self multi-headed attention (decode step).

Problem: B=32, QLEN=1, DM=1024, H=16, DK=64, TCACHE=4096, fp32.
  out = MHA(q; KV cache) with QKV projections, cache append, softmax, out-proj.

Sharding (8 NeuronCores): tensor-parallel over heads. Core c owns heads
[2c, 2c+1]: column-parallel wq/wk/wv (128 output dims per core), the KV cache
shards naturally on the head dim (134 MB/core), row-parallel wo giving a
partial [32, 1024] output per core; the host sums the 8 partials (all-reduce
done on host since the output is tiny).

Per-core kernel (memory-bound; streams 134 MB of KV cache):
  phase 0: Q^T/Knew^T/Vnew^T = W^T-chunks @ q^T-chunks on PE (+bias via ACT),
           Q bounced to DRAM for per-(batch,head) broadcast loads.
  per batch b (32 iterations, fully unrolled, double-buffered):
    - DMA K[b] and V[b] (2 MB each, contiguous) -> SBUF [128, 64, 64]
      (partition p holds t-rows {(p%64)*64 .. +63} of head p//64)
    - DVE: prod = K * q_broadcast ; segmented reduce over d -> scores [128,64]
    - ACT: e = exp(scores/8) fused with per-partition denom partial sums
    - PE: 64 accumulating matmuls per head (V-slice stationary [64t,64d],
      e-column moving [64t,1]) -> x^T column in PSUM [128, 32]
  epilogue: new-token (cache append) contribution via small PE/DVE ops,
  softmax denominator (ones-matmul partition reduce + reciprocal), x^T scaled,
  out^T = woT-chunks @ x^T on PE (+bo/8 bias) -> DRAM [128, 256].

Softmax skips the max-subtraction: scores ~ N(0,1) here, exp is safe in fp32
and the result is mathematically identical to the reference.
"""

import numpy as np
from contextlib import ExitStack

import concourse.bass as bass
import concourse.tile as tile
from concourse import bacc, mybir
from concourse.bass_utils import run_bass_kernel_spmd

F32 = mybir.dt.float32
AX = mybir.AxisListType
ALU = mybir.AluOpType
ACTF = mybir.ActivationFunctionType

B = 32          # batch
DM = 1024       # model dim
H = 16          # total heads
DK = 64         # head dim
T = 4096        # cache length
NCORES = 8
HPC = H // NCORES   # 2 heads per core
HD = HPC * DK       # 128 per-core head dims
NCH = DM // 128     # 8 contraction chunks
R = 64              # t-rows per partition in a K/V batch tile

KV_BUFS = 4         # K/V tile double-buffer depth


def _build_nc():
    nc = bacc.Bacc(
        "TRN2",
        target_bir_lowering=False,
        debug=False,
        enable_asserts=False,
        num_devices=NCORES,
    )

    qT8 = nc.dram_tensor("qT8", [128, NCH, B], F32, kind="ExternalInput").ap()
    wq8 = nc.dram_tensor("wq8", [128, NCH, HD], F32, kind="ExternalInput").ap()
    wk8 = nc.dram_tensor("wk8", [128, NCH, HD], F32, kind="ExternalInput").ap()
    wv8 = nc.dram_tensor("wv8", [128, NCH, HD], F32, kind="ExternalInput").ap()
    woT = nc.dram_tensor("woT", [HD, DM], F32, kind="ExternalInput").ap()
    cst = nc.dram_tensor("cst", [128, 11], F32, kind="ExternalInput").ap()
    idm = nc.dram_tensor("idm", [128, 128], F32, kind="ExternalInput").ap()
    kc = nc.dram_tensor("kc", [B, HPC, T, DK], F32, kind="ExternalInput").ap()
    vc = nc.dram_tensor("vc", [B, HPC, T, DK], F32, kind="ExternalInput").ap()
    outT = nc.dram_tensor("outT", [128, NCH * B], F32, kind="ExternalOutput").ap()

    kcf = kc.rearrange("b h t d -> b (h t d)")
    vcf = vc.rearrange("b h t d -> b (h t d)")

    with ExitStack() as ctx:
        tc = ctx.enter_context(tile.TileContext(nc))
        const = ctx.enter_context(tc.tile_pool(name="const", bufs=1))
        dramp = ctx.enter_context(tc.tile_pool(name="dram", bufs=1, space="DRAM"))
        psum = ctx.enter_context(tc.tile_pool(name="psum", bufs=1, space="PSUM"))

        # ---- constants into SBUF ----
        wq_sb = const.tile([128, NCH, HD], F32, tag="wq")
        wk_sb = const.tile([128, NCH, HD], F32, tag="wk")
        wv_sb = const.tile([128, NCH, HD], F32, tag="wv")
        wo_sb = const.tile([HD, DM], F32, tag="wo")
        qT_sb = const.tile([128, NCH, B], F32, tag="qt")
        cst_sb = const.tile([128, 11], F32, tag="cst")
        id_sb = const.tile([128, 128], F32, tag="idm")
        nc.sync.dma_start(wq_sb[:], wq8)
        nc.sync.dma_start(wk_sb[:], wk8)
        nc.sync.dma_start(wv_sb[:], wv8)
        nc.sync.dma_start(wo_sb[:], woT)
        nc.sync.dma_start(qT_sb[:], qT8)
        nc.sync.dma_start(cst_sb[:], cst)
        nc.sync.dma_start(id_sb[:], idm)

        ones_sb = const.tile([128, 1], F32, tag="ones")
        onerow_sb = const.tile([1, 64], F32, tag="onerow")
        nc.vector.memset(ones_sb[:], 1.0)
        nc.vector.memset(onerow_sb[:], 1.0)

        dpart = const.tile([128, B], F32, tag="dpart")

        # ---- phase 0: projections Q^T, Knew^T, Vnew^T  [128, B] ----
        QTp = psum.tile([128, B], F32, tag="p0")
        KTp = psum.tile([128, B], F32, tag="p1")
        VTp = psum.tile([128, B], F32, tag="p2")
        for c in range(NCH):
            st, sp = (c == 0), (c == NCH - 1)
            nc.tensor.matmul(QTp[:], wq_sb[:, c, :], qT_sb[:, c, :], start=st, stop=sp)
        for c in range(NCH):
            st, sp = (c == 0), (c == NCH - 1)
            nc.tensor.matmul(KTp[:], wk_sb[:, c, :], qT_sb[:, c, :], start=st, stop=sp)
        for c in range(NCH):
            st, sp = (c == 0), (c == NCH - 1)
            nc.tensor.matmul(VTp[:], wv_sb[:, c, :], qT_sb[:, c, :], start=st, stop=sp)

        QT_sb = const.tile([128, B], F32, tag="QT")
        KnT_sb = const.tile([128, B], F32, tag="KnT")
        VnT_sb = const.tile([128, B], F32, tag="VnT")
        nc.scalar.activation(QT_sb[:], QTp[:], ACTF.Identity, bias=cst_sb[:, 0:1], scale=1.0)
        nc.scalar.activation(KnT_sb[:], KTp[:], ACTF.Identity, bias=cst_sb[:, 1:2], scale=1.0)
        nc.scalar.activation(VnT_sb[:], VTp[:], ACTF.Identity, bias=cst_sb[:, 2:3], scale=1.0)

        # Q -> [B, HD] in DRAM scratch for per-batch broadcast loads
        Qp2 = psum.tile([B, 128], F32, tag="p3")
        nc.tensor.transpose(Qp2[:], QT_sb[:], id_sb[:])
        Q_sb = const.tile([B, 128], F32, tag="Q")
        nc.vector.tensor_copy(Q_sb[:], Qp2[:])
        qs = dramp.tile([B, HD], F32, tag="qs")
        nc.scalar.dma_start(qs[:], Q_sb[:])

        # ---- main loop over batches ----
        kpool = ctx.enter_context(tc.tile_pool(name="kp", bufs=KV_BUFS))
        vpool = ctx.enter_context(tc.tile_pool(name="vp", bufs=KV_BUFS))
        prodp = ctx.enter_context(tc.tile_pool(name="pp", bufs=2))
        qrp = ctx.enter_context(tc.tile_pool(name="qr", bufs=4))
        scp = ctx.enter_context(tc.tile_pool(name="scp", bufs=4))

        xpsum = psum.tile([128, B], F32, tag="px")

        for b in range(B):
            kt = kpool.tile([128, R, DK], F32, tag="k")
            vt = vpool.tile([128, R, DK], F32, tag="v")
            nc.sync.dma_start(kt[:], kcf[b].rearrange("(p r d) -> p r d", p=128, r=R))
            nc.sync.dma_start(vt[:], vcf[b].rearrange("(p r d) -> p r d", p=128, r=R))

            qrep = qrp.tile([128, DK], F32, tag="qr")
            # SWDGE: HWDGE rejects 0-stride partition-broadcast sources on HW
            nc.gpsimd.dma_start(qrep[0:64, :], qs[b, 0:DK].partition_broadcast(64))
            nc.gpsimd.dma_start(qrep[64:128, :], qs[b, DK:HD].partition_broadcast(64))

            prod = prodp.tile([128, R, DK], F32, tag="pr")
            nc.vector.tensor_mul(
                prod[:], kt[:], qrep[:].unsqueeze(1).broadcast_to([128, R, DK])
            )
            scr = scp.tile([128, R], F32, tag="sc")
            nc.vector.tensor_reduce(scr[:], prod[:], axis=AX.X, op=ALU.add)

            e = scp.tile([128, R], F32, tag="e")
            nc.scalar.activation(
                e[:], scr[:], ACTF.Exp, scale=0.125, accum_out=dpart[:, b : b + 1]
            )

            for r in range(R):
                st, sp = (r == 0), (r == R - 1)
                nc.tensor.matmul(
                    xpsum[0:64, b : b + 1], vt[0:64, r, :], e[0:64, r : r + 1],
                    start=st, stop=sp, tile_position=(0, 0),
                )
                nc.tensor.matmul(
                    xpsum[64:128, b : b + 1], vt[64:128, r, :], e[64:128, r : r + 1],
                    start=st, stop=sp, tile_position=(64, 64),
                )

        # ---- epilogue ----
        small = ctx.enter_context(tc.tile_pool(name="small", bufs=1))

        # new-token scores: s_new[h, b] = sum_d Q^T[.,b] * Knew^T[.,b] per head half
        # NB: concurrent row-group matmuls may not share a (bank, partition) set
        # on HW -> each half gets its own PSUM bank.
        prod2 = small.tile([128, B], F32, tag="prod2")
        nc.vector.tensor_mul(prod2[:], QT_sb[:], KnT_sb[:])
        snpA = psum.tile([1, B], F32, tag="p0")
        snpB = psum.tile([1, B], F32, tag="p1")
        nc.tensor.matmul(snpA[0:1, :], ones_sb[0:64, 0:1], prod2[0:64, :],
                         start=True, stop=True, tile_position=(0, 0))
        nc.tensor.matmul(snpB[0:1, :], ones_sb[64:128, 0:1], prod2[64:128, :],
                         start=True, stop=True, tile_position=(64, 0))
        e_new = small.tile([1, 2 * B], F32, tag="enew")
        nc.scalar.activation(e_new[0:1, 0:B], snpA[0:1, :], ACTF.Exp, scale=0.125)
        nc.scalar.activation(e_new[0:1, B : 2 * B], snpB[0:1, :], ACTF.Exp, scale=0.125)

        # broadcast e_new to [128, B] (head-half layout) and fold v_new into x
        erp = psum.tile([128, B], F32, tag="pe1")
        nc.tensor.matmul(erp[0:64, :], onerow_sb[0:1, 0:64], e_new[0:1, 0:B],
                         start=True, stop=True, tile_position=(0, 0))
        nc.tensor.matmul(erp[64:128, :], onerow_sb[0:1, 0:64], e_new[0:1, B : 2 * B],
                         start=True, stop=True, tile_position=(0, 64))
        tmp = small.tile([128, B], F32, tag="tmp")
        nc.vector.tensor_mul(tmp[:], VnT_sb[:], erp[:])
        xu = small.tile([128, B], F32, tag="xu")
        nc.vector.tensor_add(xu[:], tmp[:], xpsum[:])

        # denominator = per-head partition sums of dpart + e_new ; reciprocal
        dnpA = psum.tile([1, B], F32, tag="p2")
        dnpB = psum.tile([1, B], F32, tag="p3")
        nc.tensor.matmul(dnpA[0:1, :], ones_sb[0:64, 0:1], dpart[0:64, :],
                         start=True, stop=True, tile_position=(0, 0))
        nc.tensor.matmul(dnpB[0:1, :], ones_sb[64:128, 0:1], dpart[64:128, :],
                         start=True, stop=True, tile_position=(64, 0))
        dtot = small.tile([1, 2 * B], F32, tag="dtot")
        nc.vector.tensor_add(dtot[0:1, 0:B], dnpA[0:1, :], e_new[0:1, 0:B])
        nc.vector.tensor_add(dtot[0:1, B : 2 * B], dnpB[0:1, :], e_new[0:1, B : 2 * B])
        rcp = small.tile([1, 2 * B], F32, tag="rcp")
        nc.vector.reciprocal(rcp[0:1, :], dtot[0:1, :])

        rcpp = psum.tile([128, B], F32, tag="pe1")
        nc.tensor.matmul(rcpp[0:64, :], onerow_sb[0:1, 0:64], rcp[0:1, 0:B],
                         start=True, stop=True, tile_position=(0, 0))
        nc.tensor.matmul(rcpp[64:128, :], onerow_sb[0:1, 0:64], rcp[0:1, B : 2 * B],
                         start=True, stop=True, tile_position=(0, 64))
        xn = small.tile([128, B], F32, tag="xn")
        nc.vector.tensor_mul(xn[:], xu[:], rcpp[:])

        # output projection: out^T chunks [128, B] = woT-chunk.T @ x^T (+ bo/8).
        # Ping-pong PSUM banks so MM of chunk m+1 never writes the bank ACT is
        # reading (same-bank PE-W || ACT-R is a fatal PSUM collision on HW).
        outpool = ctx.enter_context(tc.tile_pool(name="pop", bufs=2, space="PSUM"))
        outsb = small.tile([128, NCH * B], F32, tag="out")
        for m in range(NCH):
            op = outpool.tile([128, B], F32, tag="po")
            nc.tensor.matmul(op[:], wo_sb[:, m * 128 : (m + 1) * 128], xn[:],
                             start=True, stop=True)
            nc.scalar.activation(outsb[:, m * B : (m + 1) * B], op[:],
                                 ACTF.Identity, bias=cst_sb[:, 3 + m : 4 + m], scale=1.0)
        nc.sync.dma_start(outT, outsb[:])

    nc.compile()
    return nc


_NC_CACHE = None


def _get_nc():
    global _NC_CACHE
    if _NC_CACHE is None:
        _NC_CACHE = _build_nc()
    return _NC_CACHE


def make_in_maps(q, key_pre, value_pre, wq, bq, wk, bk, wv, bv, wo, bo):
    q = np.asarray(q, np.float32)
    key_pre = np.asarray(key_pre, np.float32)
    value_pre = np.asarray(value_pre, np.float32)
    wq, bq = np.asarray(wq, np.float32), np.asarray(bq, np.float32)
    wk, bk = np.asarray(wk, np.float32), np.asarray(bk, np.float32)
    wv, bv = np.asarray(wv, np.float32), np.asarray(bv, np.float32)
    wo, bo = np.asarray(wo, np.float32), np.asarray(bo, np.float32)

    q2 = q.reshape(B, DM)
    qT8 = np.ascontiguousarray(q2.T.reshape(NCH, 128, B).transpose(1, 0, 2))
    idm = np.eye(128, dtype=np.float32)
    bo8 = (bo / NCORES).reshape(NCH, 128).T  # [128, 8]

    in_maps = []
    for c in range(NCORES):
        hs = slice(c * HD, (c + 1) * HD)
        heads = slice(c * HPC, (c + 1) * HPC)
        cstv = np.zeros((128, 11), np.float32)
        cstv[:, 0] = bq[hs]
        cstv[:, 1] = bk[hs]
        cstv[:, 2] = bv[hs]
        cstv[:, 3:11] = bo8
        in_maps.append({
            "qT8": qT8,
            "wq8": np.ascontiguousarray(wq[hs].T.reshape(NCH, 128, HD).transpose(1, 0, 2)),
            "wk8": np.ascontiguousarray(wk[hs].T.reshape(NCH, 128, HD).transpose(1, 0, 2)),
            "wv8": np.ascontiguousarray(wv[hs].T.reshape(NCH, 128, HD).transpose(1, 0, 2)),
            "woT": np.ascontiguousarray(wo[:, hs].T),
            "cst": cstv,
            "idm": idm,
            "kc": np.ascontiguousarray(key_pre[:, heads]),
            "vc": np.ascontiguousarray(value_pre[:, heads]),
        })
    return in_maps


def gather_output(results):
    total = np.zeros((B, DM), np.float64)
    for c in range(NCORES):
        r = results[c]["outT"]  # [128, NCH*B]
        x = r.reshape(128, NCH, B).transpose(2, 1, 0).reshape(B, DM)
        total += x
    return total.astype(np.float32).reshape(B, 1, DM)


def run(in_maps, trace=False, **kw):
    nc = _get_nc()
    return run_bass_kernel_spmd(nc, in_maps, core_ids=list(range(NCORES)),
                                trace=trace, **kw)


def kernel(q, key_pre, value_pre, wq, bq, wk, bk, wv, bv, wo, bo):
    in_maps = make_in_maps(q, key_pre, value_pre, wq, bq, wk, bk, wv, bv, wo, bo)
    res = run(in_maps, trace=False)
    return gather_output(res.results)
